# revision 7
# baseline (speedup 1.0000x reference)
"""MoE-routing kernel for Trainium2 (8 NeuronCores, data-parallel over batch).

Reference computation (per batch element b):
    q = x[b].reshape(C, H*W).T            # [S, C] rows = spatial positions
    att = q @ mempool.T                   # [S, NITEM]
    vals, idx = top_k(att, 8); w = softmax(vals)
    out = scatter(w, idx) @ mempool       # [S, C]
    y[b] = out.T.reshape(C, H, W)

Per-core plan (4 batches x 8 row-tiles of 128 rows), software-pipelined by
half-batch (4 tiles): while PE computes att scores for half h, the SWDGE
gather for half h-1 streams the selected mempool rows, and PE then runs
half h-1's combine matmuls.

    att   : PE f32r matmuls, lhsT = x chunks [c,s] (native layout), rhs = mempool.T
    top-8 : DVE max / max_index on [128, 2048]
    softmax on [128, 8] (ACT exp(+accum) + DVE recip/mul)
    gather: one SWDGE dma_gather per half (4096 rows x 1KB bf16)
    combine: PE 8 accumulating matmuls  diag(w_j) @ G_j  -> out [rows, C]
    transpose: PE transpose-mode -> outT [C, rows] -> DMA to y
"""
import numpy as np
import ml_dtypes

import concourse.bacc as bacc
import concourse.tile as tile
import concourse.mybir as mybir
from concourse import bass
from concourse.bass_utils import run_bass_kernel_spmd
from concourse.masks import make_identity

P = 128
C = 512
NITEM = 2048
K = 8
NCORES = 8
B_LOCAL = 4          # batches per core (32 / 8)
S = 1024             # spatial positions per batch (32*32)
HALF = 512           # spatial positions staged per pipeline stage
TPH = HALF // P      # tiles per half = 4

_nc_cache = {}


def build_nc(sim_compat=False):
    key = ("nc", sim_compat)
    if key in _nc_cache:
        return _nc_cache[key]
    nc = bacc.Bacc("TRN2", target_bir_lowering=False, debug=False)
    x_d = nc.dram_tensor("x", [B_LOCAL, C, S], mybir.dt.float32r, kind="ExternalInput")
    mpT_d = nc.dram_tensor("mpT", [C, NITEM], mybir.dt.float32r, kind="ExternalInput")
    mpbf_d = nc.dram_tensor("mp_bf", [NITEM, C], mybir.dt.bfloat16, kind="ExternalInput")
    y_d = nc.dram_tensor("y", [B_LOCAL, C, S], mybir.dt.float32, kind="ExternalOutput")

    with tile.TileContext(nc) as tc:
        with (
            tc.tile_pool(name="const", bufs=1) as cb,
            tc.tile_pool(name="xpool", bufs=2) as xp,
            tc.tile_pool(name="attp", bufs=3) as ap_,
            tc.tile_pool(name="small", bufs=4) as sp,
            tc.tile_pool(name="idxp", bufs=2) as ip,
            tc.tile_pool(name="diagp", bufs=8) as dp,
            tc.tile_pool(name="gpool", bufs=2) as gp,
            tc.tile_pool(name="opool", bufs=3) as op,
            tc.tile_pool(name="psL", bufs=1, space="PSUM") as psL,
            tc.tile_pool(name="psH", bufs=1, space="PSUM") as psH,
            tc.tile_pool(name="psB", bufs=2, space="PSUM") as psB,
            tc.tile_pool(name="psC", bufs=2, space="PSUM") as psC,
        ):
            # ---- one-time constants ----
            mpT_sb = cb.tile([P, 4, NITEM], mybir.dt.float32r)
            nc.sync.dma_start(mpT_sb[:], mpT_d[:].rearrange("(k p) n -> p k n", p=P))
            ident_f = cb.tile([P, P], mybir.dt.float32)
            make_identity(nc, ident_f[:])
            ident_bf = cb.tile([P, P], mybir.dt.bfloat16)
            nc.vector.tensor_copy(ident_bf[:], ident_f[:])

            def emit_att_phase(b, h):
                """Scores + top-8 + softmax + gather kickoff for half (b, h)."""
                x_sb = xp.tile([P, 4, HALF], mybir.dt.float32r, tag="x")
                nc.sync.dma_start(
                    x_sb[:],
                    x_d[b].rearrange("(kc c) s -> c kc s", c=P)[:, :, h * HALF:(h + 1) * HALF],
                )
                idx_half = ip.tile([P, TPH, K], mybir.dt.uint16, tag="idx")
                vals_half = ip.tile([P, TPH, K], mybir.dt.float32, tag="vals")
                diags = []
                for tt in range(TPH):
                    att_sb = ap_.tile([P, NITEM], mybir.dt.float32, tag="attsb")
                    # items 0..1023 -> psL, 1024..2047 -> psH; copy lo while hi runs
                    for half_i, pool in ((0, psL), (1, psH)):
                        att_ps = pool.tile([P, NITEM // 2], mybir.dt.float32,
                                           space="PSUM", tag=f"att{half_i}")
                        for k in range(4):
                            for n in range(2):
                                nc.tensor.matmul(
                                    att_ps[:, n * 512:(n + 1) * 512],
                                    lhsT=x_sb[:, k, tt * P:(tt + 1) * P],
                                    rhs=mpT_sb[:, k, (2 * half_i + n) * 512:(2 * half_i + n + 1) * 512],
                                    start=(k == 0),
                                    stop=(k == 3),
                                )
                        nc.scalar.copy(
                            att_sb[:, half_i * (NITEM // 2):(half_i + 1) * (NITEM // 2)],
                            att_ps[:],
                        )

                    nc.vector.max(out=vals_half[:, tt, :], in_=att_sb[:])
                    nc.vector.max_index(
                        out=idx_half[:, tt, :],
                        in_max=vals_half[:, tt, :],
                        in_values=att_sb[:],
                    )
                    # softmax over the 8 vals (no max-shift; |att| < ~6)
                    e8 = sp.tile([P, K], mybir.dt.float32, tag="e8")
                    z = sp.tile([P, 1], mybir.dt.float32, tag="z")
                    nc.scalar.activation(e8[:], vals_half[:, tt, :],
                                         mybir.ActivationFunctionType.Exp, accum_out=z[:])
                    rz = sp.tile([P, 1], mybir.dt.float32, tag="rz")
                    nc.vector.reciprocal(rz[:], z[:])
                    w8 = sp.tile([P, K], mybir.dt.float32, tag="w8")
                    nc.vector.tensor_scalar_mul(w8[:], e8[:], rz[:, :1])
                    diag = dp.tile([P, K, P], mybir.dt.bfloat16, tag="diag")
                    for j in range(K):
                        nc.vector.tensor_scalar_mul(diag[:, j, :], ident_bf[:], w8[:, j:j + 1])
                    diags.append(diag)

                # wrap: flat k = (tile*8 + j)*128 + p lives at [grp*16 + k%16, k//16]
                # W[16g+q, (tile*8+j)*8 + t] = idx_half[16t+q, tile, j]
                W = ip.tile([P, TPH * 64], mybir.dt.int16, tag="W")
                nc.gpsimd.memset(W[:], 0)
                idx_i16 = idx_half[:].bitcast(mybir.dt.int16)
                groups = (0, 1) if sim_compat else (1,)
                for g_ in groups:
                    for t in range(8):
                        nc.sync.dma_start(
                            W[16 * g_:16 * (g_ + 1)]
                            .rearrange("q (tile j t) -> q tile j t", tile=TPH, j=K)[:, :, :, t],
                            idx_i16[16 * t:16 * (t + 1)],
                        )
                G = gp.tile([P, TPH * K, C], mybir.dt.bfloat16, tag="g")
                for tt in range(TPH):
                    nc.gpsimd.dma_gather(
                        out_ap=G[:, tt * K:(tt + 1) * K, :], in_ap=mpbf_d[:],
                        idxs_ap=W[:, tt * 64:(tt + 1) * 64],
                        num_idxs=P * K, num_idxs_reg=P * K, elem_size=C,
                    )
                return (b, h, G, diags)

            def emit_combine(stage):
                """Combine + transpose + store for a previously-scored half."""
                b, h, G, diags = stage
                pend = []  # transposes trail combines by one tile

                def emit_transpose(tt, out_sb):
                    s0 = h * HALF + tt * P
                    outT_ps = psC.tile([P, C], mybir.dt.float32, space="PSUM", tag="outT")
                    for cc in range(4):
                        nc.tensor.transpose(
                            outT_ps[:, cc * P:(cc + 1) * P],
                            in_=out_sb[:, cc * P:(cc + 1) * P],
                            identity=ident_f[:],
                        )
                    outT_sb = op.tile([P, 4, P], mybir.dt.float32, tag="outTsb")
                    nc.scalar.copy(outT_sb[:], outT_ps[:].rearrange("p (cc s) -> p cc s", cc=4))
                    nc.sync.dma_start(
                        y_d[b].rearrange("(cc c) s -> c cc s", c=P)[:, :, s0:s0 + P],
                        outT_sb[:],
                    )

                for tt in range(TPH):
                    diag = diags[tt]
                    out_ps = psB.tile([P, C], mybir.dt.float32, space="PSUM", tag="out")
                    for j in range(K):
                        nc.tensor.matmul(
                            out_ps[:],
                            lhsT=diag[:, j, :],
                            rhs=G[:, tt * K + j, :],
                            start=(j == 0),
                            stop=(j == K - 1),
                        )
                    out_sb = op.tile([P, C], mybir.dt.float32, tag="outsb")
                    nc.scalar.copy(out_sb[:], out_ps[:])
                    pend.append((tt, out_sb))
                    if len(pend) > 1:
                        emit_transpose(*pend.pop(0))
                while pend:
                    emit_transpose(*pend.pop(0))

            prev = None
            for b in range(B_LOCAL):
                for h in range(S // HALF):
                    stage = emit_att_phase(b, h)
                    if prev is not None:
                        emit_combine(prev)
                    prev = stage
            emit_combine(prev)
    nc.compile()
    _nc_cache[key] = nc
    return nc


def _prep_in_maps(x, mempool):
    xs = np.ascontiguousarray(x.reshape(NCORES, B_LOCAL, C, S))
    mpT = np.ascontiguousarray(mempool.T)
    mp_bf = np.ascontiguousarray(mempool.astype(ml_dtypes.bfloat16))
    return [
        {"x": xs[c], "mpT": mpT, "mp_bf": mp_bf}
        for c in range(NCORES)
    ]


def kernel(x, mempool, k):
    assert int(k) == K
    x = np.asarray(x, dtype=np.float32)
    mempool = np.asarray(mempool, dtype=np.float32)
    B, Cx, H, Wd = x.shape
    assert (B, Cx, H * Wd) == (32, C, S) and mempool.shape == (NITEM, C)

    nc = build_nc()
    in_maps = _prep_in_maps(x, mempool)
    res = run_bass_kernel_spmd(nc, in_maps, core_ids=list(range(NCORES)))
    y = np.stack([res.results[c]["y"] for c in range(NCORES)])  # [8, 4, C, S]
    return np.ascontiguousarray(y.reshape(B, C, H, Wd))
